# revision 21
# baseline (speedup 1.0000x reference)
"""Trainium2 Bass kernel for nn_MultiHeadAttention_85761906966848 (sparse_attention).

The reference module only uses the DIAGONAL of the softmax attention matrix:
    out[b,s,:] = (softmax(masked scores)[s,s] * v[b,s,:]) @ W0 + b0
so no attn @ V matmul is needed — only QK^T row-sums of exp (softmax
denominators), the diagonal q_s.k_s, and the four dense projections.

Sharding: TENSOR-PARALLEL over heads. Core j owns heads (2j, 2j+1):
  * Q/K/V projections restricted to that 128-wide feature slice for ALL
    batches, sequence axis trimmed to ceil(L_b/128)*128 valid rows.
  * Scores / softmax denominators / diagonal weights per local head.
  * O-projection uses the 128-row slice of W0 -> per-core PARTIAL outputs,
    summed on the host (linear combine), which also adds b0.
Uniform SPMD by construction; only the weight slices per in_map differ.

Matmuls in bf16; causal mask folded into the score matmul as an extra
(-1e30*I).T @ strict_upper_ones PSUM accumulation. Score tiles of width<=512
pack BOTH heads into one PSUM tile: one exp + one strided 3-D reduce per
pair, halving the scalar-engine op count on the critical softmax path.
Epilogue work of batch b-1 is woven between score tiles of batch b.
"""

import numpy as np
import ml_dtypes
import concourse.bass as bass
import concourse.bacc as bacc
import concourse.mybir as mybir
from concourse import tile

F32 = mybir.dt.float32
BF16 = mybir.dt.bfloat16
AF = mybir.ActivationFunctionType
AX = mybir.AxisListType
ALU = mybir.AluOpType

B, S, D, H = 8, 1024, 1024, 16
dk = D // H
HL = 2
NEG = -1.0e30

_CACHE = {}


def blocks(total, width):
    out = []
    off = 0
    while off < total:
        w = min(width, total - off)
        out.append((off, w))
        off += w
    return out


def _build(lts):
    """lts: tuple of per-batch 128-row tile counts, in processing order."""
    NB = len(lts)
    TC = sum(lts)
    SC = TC * 128
    offs = []
    o = 0
    for lt in lts:
        offs.append(o * 128)
        o += lt
    C = D // 128

    nc = bacc.Bacc("TRN2", target_bir_lowering=False, debug=False, num_devices=8)

    xt_d = nc.dram_tensor("xt", [C, 128, SC], BF16, kind="ExternalInput")
    wq_d = nc.dram_tensor("wq", [C, 128, 128], BF16, kind="ExternalInput")
    wk_d = nc.dram_tensor("wk", [C, 128, 128], BF16, kind="ExternalInput")
    wv_d = nc.dram_tensor("wv", [C, 128, 128], BF16, kind="ExternalInput")
    w0_d = nc.dram_tensor("w0", [128, D], BF16, kind="ExternalInput")
    bq_d = nc.dram_tensor("bq", [128, 1], F32, kind="ExternalInput")
    bk_d = nc.dram_tensor("bk", [128, 1], F32, kind="ExternalInput")
    bv_d = nc.dram_tensor("bv", [128, 1], F32, kind="ExternalInput")
    negi_d = nc.dram_tensor("negi", [128, 128], BF16, kind="ExternalInput")
    ub_d = nc.dram_tensor("ub", [128, 128], BF16, kind="ExternalInput")
    idenf_d = nc.dram_tensor("idenf", [128, 128], F32, kind="ExternalInput")
    ones64_d = nc.dram_tensor("ones64", [128, 1], BF16, kind="ExternalInput")
    ind2_d = nc.dram_tensor("ind2", [33, 128], BF16, kind="ExternalInput")
    out_d = nc.dram_tensor("out", [SC, D], BF16, kind="ExternalOutput")

    with tile.TileContext(nc) as tc:
        with (
            tc.tile_pool(name="cp", bufs=1) as cp,
            tc.tile_pool(name="xp", bufs=1) as xp,
            tc.tile_pool(name="qkp", bufs=1) as qkp,
            tc.tile_pool(name="srp", bufs=3) as srp,
            tc.tile_pool(name="prp", bufs=3) as prp,
            tc.tile_pool(name="dlp", bufs=2) as dlp,
            tc.tile_pool(name="otp", bufs=3) as otp,
            tc.tile_pool(name="psc", bufs=3, space=bass.MemorySpace.PSUM) as psc,
            tc.tile_pool(name="pmm", bufs=2, space=bass.MemorySpace.PSUM) as pmm,
        ):
            # ---------------- constants (sync queue) ----------------
            negi = cp.tile([128, 128], BF16, tag="negi")
            nc.sync.dma_start(negi[:], negi_d[:])
            ub = cp.tile([128, 128], BF16, tag="ub")
            nc.sync.dma_start(ub[:], ub_d[:])
            idenf = cp.tile([128, 128], F32, tag="idenf")
            nc.sync.dma_start(idenf[:], idenf_d[:])
            ones64 = cp.tile([128, 1], BF16, tag="ones64")
            nc.sync.dma_start(ones64[:], ones64_d[:])
            ind2 = cp.tile([33, 128], BF16, tag="ind2")
            nc.sync.dma_start(ind2[:], ind2_d[:])
            bq = cp.tile([128, 1], F32, tag="bq")
            nc.sync.dma_start(bq[:], bq_d[:])
            bk = cp.tile([128, 1], F32, tag="bk")
            nc.sync.dma_start(bk[:], bk_d[:])
            bv = cp.tile([128, 1], F32, tag="bv")
            nc.sync.dma_start(bv[:], bv_d[:])

            # weights on scalar queue first (small, needed first)
            wqs = [cp.tile([128, 128], BF16, name=f"wq{c}", tag=f"wq{c}") for c in range(C)]
            wks = [cp.tile([128, 128], BF16, name=f"wk{c}", tag=f"wk{c}") for c in range(C)]
            wvs = [cp.tile([128, 128], BF16, name=f"wv{c}", tag=f"wv{c}") for c in range(C)]
            for c in range(C):
                nc.scalar.dma_start(wks[c][:], wk_d[c, :, :])
            for c in range(C):
                nc.scalar.dma_start(wqs[c][:], wq_d[c, :, :])

            # X^T resident: narrow first block for a fast start, wide after
            xt = [xp.tile([128, SC], BF16, name=f"xt{c}", tag=f"xt{c}") for c in range(C)]
            di = 0
            xengs = [nc.sync, nc.scalar, nc.gpsimd]
            xpieces = [(0, 512)] + [(512 + o, w) for (o, w) in blocks(SC - 512, 1024)]
            for (boff, bw) in xpieces:
                for c in range(C):
                    xengs[di % 3].dma_start(xt[c][:, boff:boff + bw],
                                            xt_d[c, :, boff:boff + bw])
                    di += 1
            for c in range(C):
                nc.scalar.dma_start(wvs[c][:], wv_d[c, :, :])
            w0s = cp.tile([128, D], BF16, tag="w0s")
            nc.scalar.dma_start(w0s[:], w0_d[:])

            # persistent per-core tensors
            qth = [qkp.tile([64, SC], BF16, name=f"qth{h}", tag=f"qth{h}") for h in range(HL)]
            kth = [qkp.tile([64, SC], BF16, name=f"kth{h}", tag=f"kth{h}") for h in range(HL)]
            vt = qkp.tile([128, SC], BF16, tag="vt")
            a2 = cp.tile([33, SC], BF16, tag="a2")
            nc.vector.memset(a2[:], 0.0)
            dn = [cp.tile([128, 2 * lt], F32, name=f"dn{b}", tag=f"dn{b}")
                  for b, lt in enumerate(lts)]

            # -------- streaming K/Q/V projection, one 512 block at a time
            def drain_k(ps, boff, bw):
                nc.scalar.activation(kth[0][:, boff:boff + bw], ps[0:64, 0:bw],
                                     AF.Identity, bias=bk[0:64, :])
                nc.vector.tensor_scalar_add(kth[1][:, boff:boff + bw],
                                            ps[64:128, 0:bw], bk[64:128, :])

            def drain_q(ps, boff, bw):
                nc.vector.tensor_scalar_add(qth[0][:, boff:boff + bw],
                                            ps[0:64, 0:bw], bq[0:64, :])
                nc.scalar.activation(qth[1][:, boff:boff + bw], ps[64:128, 0:bw],
                                     AF.Identity, bias=bq[64:128, :])

            def drain_v(ps, boff, bw):
                nc.vector.tensor_scalar_add(vt[:, boff:boff + bw], ps[:, 0:bw],
                                            bv[:])

            pblocks = blocks(SC, 512)
            pstate = {"next": 0}

            def emit_proj_block():
                boff, bw = pblocks[pstate["next"]]
                pstate["next"] += 1
                for w_tiles, drain in ((wks, drain_k), (wqs, drain_q)):
                    ps = pmm.tile([128, 512], F32, tag="mm")
                    for kk in range(C):
                        nc.tensor.matmul(ps[:, 0:bw], w_tiles[kk][:],
                                         xt[kk][:, boff:boff + bw],
                                         start=(kk == 0), stop=(kk == C - 1))
                    drain(ps, boff, bw)

            def emit_proj_some(n):
                for _ in range(n):
                    if pstate["next"] < len(pblocks):
                        emit_proj_block()

            def emit_proj_upto(col_end):
                while (pstate["next"] < len(pblocks)
                       and pblocks[pstate["next"]][0] < col_end):
                    emit_proj_block()

            vstate = {"next": 0}

            def emit_v_block():
                boff, bw = pblocks[vstate["next"]]
                vstate["next"] += 1
                ps = pmm.tile([128, 512], F32, tag="mm")
                for kk in range(C):
                    nc.tensor.matmul(ps[:, 0:bw], wvs[kk][:],
                                     xt[kk][:, boff:boff + bw],
                                     start=(kk == 0), stop=(kk == C - 1))
                drain_v(ps, boff, bw)

            def emit_v_some(n):
                for _ in range(n):
                    if vstate["next"] < len(pblocks):
                        emit_v_block()

            def emit_v_upto(col_end):
                while (vstate["next"] < len(pblocks)
                       and pblocks[vstate["next"]][0] < col_end):
                    emit_v_block()

            # ---------------- diag products (q*k hi/lo), per 512 block -----
            prod_tiles = {}

            def emit_prod(e):
                off = offs[e]
                scb = lts[e] * 128
                for h in range(HL):
                    for bi, (boff, bw) in enumerate(blocks(scb, 512)):
                        sli = slice(off + boff, off + boff + bw)
                        pr32 = prp.tile([64, 512], F32, tag="pr32")
                        nc.vector.tensor_mul(pr32[:, 0:bw], qth[h][:, sli],
                                             kth[h][:, sli])
                        prhl = prp.tile([128, 512], BF16, name=f"prhl{h}_{bi}",
                                        tag=f"prhl{h}_{bi}", bufs=1)
                        nc.gpsimd.tensor_copy(prhl[0:64, 0:bw], pr32[:, 0:bw])
                        nc.gpsimd.tensor_sub(prhl[64:128, 0:bw], pr32[:, 0:bw],
                                             prhl[0:64, 0:bw])
                        prod_tiles[(e, h, bi)] = (prhl, bw)

            # ------------- epilogue of batch e as a unit queue -------------
            def epilogue_units(e, oeng):
                off = offs[e]
                lt = lts[e]
                scb = lt * 128
                st = {}

                def u_recip():
                    emit_v_upto(off + scb)
                    rec = prp.tile([128, 16], F32, tag="rec")
                    nc.vector.reciprocal(rec[:, 0:2 * lt], dn[e][:])
                    tpr = pmm.tile([16, 128], F32, tag="mm")
                    nc.tensor.transpose(tpr[0:2 * lt, :], rec[:, 0:2 * lt], idenf[:])
                    recT = prp.tile([16, 128], BF16, tag="recT")
                    nc.vector.tensor_copy(recT[0:2 * lt, :], tpr[0:2 * lt, :])
                    st["arecl"] = [dlp.tile([1, 1024], BF16, name=f"arecl{h}",
                                            tag=f"arecl{h}") for h in range(HL)]
                    for h in range(HL):
                        nc.sync.dma_start(st["arecl"][h][:, 0:scb],
                                          recT[h:2 * lt:2, :])
                yield u_recip

                def u_diag():
                    st["dexpl"] = [dlp.tile([1, 1024], BF16, name=f"dexpl{h}",
                                            tag=f"dexpl{h}") for h in range(HL)]
                    for h in range(HL):
                        for bi, (boff, bw) in enumerate(blocks(scb, 512)):
                            prhl, _ = prod_tiles.pop((e, h, bi))
                            dg = pmm.tile([1, 512], F32, tag="mm")
                            nc.tensor.matmul(dg[:, 0:bw], ones64[:],
                                             prhl[:, 0:bw],
                                             start=True, stop=True)
                            nc.scalar.activation(st["dexpl"][h][:, boff:boff + bw],
                                                 dg[:, 0:bw], AF.Exp)
                yield u_diag

                def u_a2():
                    for h in range(HL):
                        nc.vector.tensor_mul(a2[32 * h:32 * h + 1, off:off + scb],
                                             st["dexpl"][h][:, 0:scb],
                                             st["arecl"][h][:, 0:scb])
                yield u_a2

                for (boff, bw) in blocks(scb, 512):
                    def u_ab(boff=boff, bw=bw):
                        ab = pmm.tile([128, 512], F32, tag="mm")
                        nc.tensor.matmul(ab[:, 0:bw], ind2[:],
                                         a2[:, off + boff: off + boff + bw],
                                         start=True, stop=True)
                        nc.vector.tensor_mul(vt[:, off + boff: off + boff + bw],
                                             vt[:, off + boff: off + boff + bw],
                                             ab[:, 0:bw])
                    yield u_ab

                for i in range(lt):
                    def u_out(i=i):
                        cs = off + i * 128
                        ot = otp.tile([128, 1024], BF16, tag="ot")
                        for oi, (ooff, ow) in enumerate(blocks(D, 512)):
                            po = pmm.tile([128, 512], F32, tag="mm")
                            nc.tensor.matmul(po[:, 0:ow], vt[:, cs:cs + 128],
                                             w0s[:, ooff:ooff + ow],
                                             start=True, stop=True)
                            eng = oeng[0]
                            oeng[0] = (oeng[0] + 1) % 2
                            if eng == 0:
                                nc.vector.tensor_copy(ot[:, ooff:ooff + ow],
                                                      po[:, 0:ow])
                            else:
                                nc.scalar.copy(ot[:, ooff:ooff + ow], po[:, 0:ow])
                        nc.sync.dma_start(out_d[cs:cs + 128, :], ot[:])
                    yield u_out

            # ---------------- scores with woven epilogue ----------------
            def emit_scores(b, pending):
                off = offs[b]
                lt = lts[b]
                emit_proj_upto(off + lt * 128)
                for i in range(lt):
                    emit_proj_some(1)
                    emit_v_some(1)
                    npop = max(3, -(-len(pending) // max(1, lt - i)))
                    N = (i + 1) * 128
                    if N <= 512:
                        # pack both heads in one PSUM tile at 512-aligned slot
                        # offsets (a matmul write must not cross a PSUM bank
                        # boundary): one exp + one 3-D reduce for the pair
                        w = N
                        sc = psc.tile([128, 1024], F32, tag="sc")
                        sc3 = sc.rearrange("p (s c) -> p s c", s=2)
                        for h in range(HL):
                            so = h * 512
                            nc.tensor.matmul(sc[:, so:so + w],
                                             qth[h][:, off + i * 128: off + N],
                                             kth[h][:, off: off + w],
                                             start=True, stop=False)
                            nc.tensor.matmul(sc[:, so + w - 128: so + w],
                                             negi[:], ub[:],
                                             start=False, stop=True,
                                             skip_group_check=True)
                        scr = srp.tile([128, 2, 512], BF16, tag="scr")
                        nc.scalar.activation(scr[:, 0:2, 0:w], sc3[:, 0:2, 0:w],
                                             AF.Exp)
                        nc.vector.tensor_reduce(dn[b][:, 2 * i: 2 * i + 2],
                                                scr[:, 0:2, 0:w], AX.X, ALU.add)
                    else:
                        for h in range(HL):
                            sc = psc.tile([128, 1024], F32, tag="sc")
                            bl = blocks(N, 512)
                            for bi, (boff, bw) in enumerate(bl):
                                last = (bi == len(bl) - 1)
                                nc.tensor.matmul(
                                    sc[:, boff:boff + bw],
                                    qth[h][:, off + i * 128: off + N],
                                    kth[h][:, off + boff: off + boff + bw],
                                    start=True, stop=not last)
                            nc.tensor.matmul(sc[:, N - 128:N], negi[:], ub[:],
                                             start=False, stop=True,
                                             skip_group_check=True)
                            scr = srp.tile([128, 1024], BF16, tag="scrw")
                            nc.scalar.activation(scr[:, 0:N], sc[:, 0:N], AF.Exp,
                                                 accum_out=dn[b][:, 2 * i + h:
                                                                 2 * i + h + 1])
                    for _ in range(npop):
                        if pending:
                            pending.pop(0)()
                emit_prod(b)

            # ---------------- main schedule ----------------
            oeng = [0]
            pending = []
            for b in range(NB):
                emit_scores(b, pending)
                while pending:
                    pending.pop(0)()
                pending = list(epilogue_units(b, oeng))
            while pending:
                pending.pop(0)()

    nc.compile()
    return nc


def _get_nc(lts):
    key = tuple(lts)
    if key not in _CACHE:
        _CACHE[key] = _build(key)
    return _CACHE[key]


def _host_consts():
    aux = {}
    negi = np.zeros((128, 128), np.float32)
    np.fill_diagonal(negi, NEG)
    aux["negi"] = negi.astype(ml_dtypes.bfloat16)
    aux["ub"] = np.triu(np.ones((128, 128), np.float32), 1).astype(ml_dtypes.bfloat16)
    aux["idenf"] = np.eye(128, dtype=np.float32)
    aux["ones64"] = np.ones((128, 1), np.float32).astype(ml_dtypes.bfloat16)
    ind2 = np.zeros((33, 128), np.float32)
    ind2[0, 0:64] = 1.0
    ind2[32, 64:128] = 1.0
    aux["ind2"] = ind2.astype(ml_dtypes.bfloat16)
    return aux


def _run(inputs, trace=False):
    from concourse.bass_utils import run_bass_kernel_spmd

    batch = np.asarray(inputs["batch"], np.float32)
    lengths = np.asarray(inputs["lengths"]).astype(np.int64)
    assert batch.shape == (B, S, D), batch.shape
    lt_all = [max(1, int(np.ceil(int(l) / 128.0))) for l in lengths]
    order = sorted(range(B), key=lambda b: -lt_all[b])
    lts = tuple(lt_all[b] for b in order)
    offs = []
    o = 0
    for lt in lts:
        offs.append(o * 128)
        o += lt
    SC = o * 128

    nc = _get_nc(lts)

    XT = np.concatenate(
        [batch[order[k]][: lts[k] * 128, :].T for k in range(B)], axis=1)
    xt = np.ascontiguousarray(XT.reshape(D // 128, 128, SC)).astype(ml_dtypes.bfloat16)
    consts = _host_consts()
    wq = np.asarray(inputs["wq"], np.float32)
    wk = np.asarray(inputs["wk"], np.float32)
    wv = np.asarray(inputs["wv"], np.float32)
    w0 = np.asarray(inputs["w0"], np.float32)
    bqf = np.asarray(inputs["bq"], np.float32)
    bkf = np.asarray(inputs["bk"], np.float32)
    bvf = np.asarray(inputs["bv"], np.float32)

    in_maps = []
    for j in range(8):
        sl = slice(j * 128, (j + 1) * 128)
        im = dict(consts)
        im["xt"] = xt
        im["wq"] = np.ascontiguousarray(
            wq[:, sl].reshape(8, 128, 128)).astype(ml_dtypes.bfloat16)
        im["wk"] = np.ascontiguousarray(
            wk[:, sl].reshape(8, 128, 128)).astype(ml_dtypes.bfloat16)
        im["wv"] = np.ascontiguousarray(
            wv[:, sl].reshape(8, 128, 128)).astype(ml_dtypes.bfloat16)
        im["w0"] = np.ascontiguousarray(w0[sl, :]).astype(ml_dtypes.bfloat16)
        im["bq"] = np.ascontiguousarray(bqf[sl].reshape(128, 1))
        im["bk"] = np.ascontiguousarray(bkf[sl].reshape(128, 1))
        im["bv"] = np.ascontiguousarray(bvf[sl].reshape(128, 1))
        in_maps.append(im)

    res = run_bass_kernel_spmd(nc, in_maps, core_ids=list(range(8)), trace=trace)

    acc = np.zeros((SC, D), np.float32)
    for r in res.results:
        acc += np.asarray(r["out"]).astype(np.float32)
    b0 = np.asarray(inputs["b0"], np.float32)
    out = np.empty((B, S, D), np.float32)
    out[:] = b0[None, None, :]
    for k in range(B):
        b = order[k]
        L = int(lengths[b])
        out[b, :L, :] += acc[offs[k]: offs[k] + L, :]
    return out, res


def kernel(**inputs) -> np.ndarray:
    out, _ = _run(inputs, trace=False)
    return out


# revision 22
# speedup vs baseline: 1.1706x; 1.1706x over previous
"""Trainium2 Bass kernel for nn_MultiHeadAttention_85761906966848 (sparse_attention).

The reference module only uses the DIAGONAL of the softmax attention matrix:
    out[b,s,:] = (softmax(masked scores)[s,s] * v[b,s,:]) @ W0 + b0
so no attn @ V matmul is needed — only QK^T row-sums of exp (softmax
denominators), the diagonal q_s.k_s, and the four dense projections.

Sharding: TENSOR-PARALLEL over heads. Core j owns heads (2j, 2j+1):
  * Q/K/V projections restricted to that 128-wide feature slice for ALL
    batches, sequence axis trimmed to ceil(L_b/128)*128 valid rows.
  * Scores / softmax denominators / diagonal weights per local head.
  * O-projection uses the 128-row slice of W0 -> per-core PARTIAL outputs,
    summed on the host (linear combine), which also adds b0.
Uniform SPMD by construction; only the weight slices per in_map differ.

Matmuls in bf16; causal mask folded into the score matmul as an extra
(-1e30*I).T @ strict_upper_ones PSUM accumulation. Score tiles of width<=512
pack BOTH heads into one PSUM tile: one exp + one strided 3-D reduce per
pair, halving the scalar-engine op count on the critical softmax path.
Epilogue work of batch b-1 is woven between score tiles of batch b.
"""

import numpy as np
import ml_dtypes
import concourse.bass as bass
import concourse.bacc as bacc
import concourse.mybir as mybir
from concourse import tile

F32 = mybir.dt.float32
BF16 = mybir.dt.bfloat16
AF = mybir.ActivationFunctionType
AX = mybir.AxisListType
ALU = mybir.AluOpType

B, S, D, H = 8, 1024, 1024, 16
dk = D // H
HL = 2
NEG = -1.0e30

_CACHE = {}


def blocks(total, width):
    out = []
    off = 0
    while off < total:
        w = min(width, total - off)
        out.append((off, w))
        off += w
    return out


def _build(lts):
    """lts: tuple of per-batch 128-row tile counts, in processing order."""
    NB = len(lts)
    TC = sum(lts)
    SC = TC * 128
    offs = []
    o = 0
    for lt in lts:
        offs.append(o * 128)
        o += lt
    C = D // 128

    nc = bacc.Bacc("TRN2", target_bir_lowering=False, debug=False, num_devices=8)

    xt_d = nc.dram_tensor("xt", [C, 128, SC], BF16, kind="ExternalInput")
    wq_d = nc.dram_tensor("wq", [C, 128, 128], BF16, kind="ExternalInput")
    wk_d = nc.dram_tensor("wk", [C, 128, 128], BF16, kind="ExternalInput")
    wv_d = nc.dram_tensor("wv", [C, 128, 128], BF16, kind="ExternalInput")
    w0_d = nc.dram_tensor("w0", [128, D], BF16, kind="ExternalInput")
    bq_d = nc.dram_tensor("bq", [128, 1], F32, kind="ExternalInput")
    bk_d = nc.dram_tensor("bk", [128, 1], F32, kind="ExternalInput")
    bv_d = nc.dram_tensor("bv", [128, 1], F32, kind="ExternalInput")
    negi_d = nc.dram_tensor("negi", [128, 128], BF16, kind="ExternalInput")
    ub_d = nc.dram_tensor("ub", [128, 128], BF16, kind="ExternalInput")
    idenf_d = nc.dram_tensor("idenf", [128, 128], F32, kind="ExternalInput")
    ones64_d = nc.dram_tensor("ones64", [64, 1], BF16, kind="ExternalInput")
    ind2_d = nc.dram_tensor("ind2", [33, 128], BF16, kind="ExternalInput")
    out_d = nc.dram_tensor("out", [SC, D], BF16, kind="ExternalOutput")

    with tile.TileContext(nc) as tc:
        with (
            tc.tile_pool(name="cp", bufs=1) as cp,
            tc.tile_pool(name="xp", bufs=1) as xp,
            tc.tile_pool(name="qkp", bufs=1) as qkp,
            tc.tile_pool(name="srp", bufs=3) as srp,
            tc.tile_pool(name="prp", bufs=3) as prp,
            tc.tile_pool(name="dlp", bufs=2) as dlp,
            tc.tile_pool(name="otp", bufs=3) as otp,
            tc.tile_pool(name="psc", bufs=3, space=bass.MemorySpace.PSUM) as psc,
            tc.tile_pool(name="pmm", bufs=2, space=bass.MemorySpace.PSUM) as pmm,
        ):
            # ---------------- constants (sync queue) ----------------
            negi = cp.tile([128, 128], BF16, tag="negi")
            nc.sync.dma_start(negi[:], negi_d[:])
            ub = cp.tile([128, 128], BF16, tag="ub")
            nc.sync.dma_start(ub[:], ub_d[:])
            idenf = cp.tile([128, 128], F32, tag="idenf")
            nc.sync.dma_start(idenf[:], idenf_d[:])
            ones64 = cp.tile([64, 1], BF16, tag="ones64")
            nc.sync.dma_start(ones64[:], ones64_d[:])
            ind2 = cp.tile([33, 128], BF16, tag="ind2")
            nc.sync.dma_start(ind2[:], ind2_d[:])
            bq = cp.tile([128, 1], F32, tag="bq")
            nc.sync.dma_start(bq[:], bq_d[:])
            bk = cp.tile([128, 1], F32, tag="bk")
            nc.sync.dma_start(bk[:], bk_d[:])
            bv = cp.tile([128, 1], F32, tag="bv")
            nc.sync.dma_start(bv[:], bv_d[:])

            # weights on scalar queue first (small, needed first)
            wqs = [cp.tile([128, 128], BF16, name=f"wq{c}", tag=f"wq{c}") for c in range(C)]
            wks = [cp.tile([128, 128], BF16, name=f"wk{c}", tag=f"wk{c}") for c in range(C)]
            wvs = [cp.tile([128, 128], BF16, name=f"wv{c}", tag=f"wv{c}") for c in range(C)]
            for c in range(C):
                nc.scalar.dma_start(wks[c][:], wk_d[c, :, :])
            for c in range(C):
                nc.scalar.dma_start(wqs[c][:], wq_d[c, :, :])

            # X^T resident: narrow first block for a fast start, wide after
            xt = [xp.tile([128, SC], BF16, name=f"xt{c}", tag=f"xt{c}") for c in range(C)]
            di = 0
            xengs = [nc.sync, nc.scalar, nc.gpsimd]
            for (boff, bw) in blocks(SC, 1024):
                for c in range(C):
                    xengs[di % 3].dma_start(xt[c][:, boff:boff + bw],
                                            xt_d[c, :, boff:boff + bw])
                    di += 1
            for c in range(C):
                nc.scalar.dma_start(wvs[c][:], wv_d[c, :, :])
            w0s = cp.tile([128, D], BF16, tag="w0s")
            nc.scalar.dma_start(w0s[:], w0_d[:])

            # persistent per-core tensors
            qth = [qkp.tile([64, SC], BF16, name=f"qth{h}", tag=f"qth{h}") for h in range(HL)]
            kth = [qkp.tile([64, SC], BF16, name=f"kth{h}", tag=f"kth{h}") for h in range(HL)]
            vt = qkp.tile([128, SC], BF16, tag="vt")
            a2 = cp.tile([33, SC], BF16, tag="a2")
            nc.vector.memset(a2[:], 0.0)
            dn = [cp.tile([128, 2 * lt], F32, name=f"dn{b}", tag=f"dn{b}")
                  for b, lt in enumerate(lts)]

            # -------- streaming K/Q/V projection, one 512 block at a time
            def drain_k(ps, boff, bw):
                nc.scalar.activation(kth[0][:, boff:boff + bw], ps[0:64, 0:bw],
                                     AF.Identity, bias=bk[0:64, :])
                nc.vector.tensor_scalar_add(kth[1][:, boff:boff + bw],
                                            ps[64:128, 0:bw], bk[64:128, :])

            def drain_q(ps, boff, bw):
                nc.vector.tensor_scalar_add(qth[0][:, boff:boff + bw],
                                            ps[0:64, 0:bw], bq[0:64, :])
                nc.scalar.activation(qth[1][:, boff:boff + bw], ps[64:128, 0:bw],
                                     AF.Identity, bias=bq[64:128, :])

            def drain_v(ps, boff, bw):
                nc.vector.tensor_scalar_add(vt[:, boff:boff + bw], ps[:, 0:bw],
                                            bv[:])

            pblocks = blocks(SC, 512)
            pstate = {"next": 0}

            def emit_proj_block():
                boff, bw = pblocks[pstate["next"]]
                pstate["next"] += 1
                for w_tiles, drain in ((wks, drain_k), (wqs, drain_q)):
                    ps = pmm.tile([128, 512], F32, tag="mm")
                    for kk in range(C):
                        nc.tensor.matmul(ps[:, 0:bw], w_tiles[kk][:],
                                         xt[kk][:, boff:boff + bw],
                                         start=(kk == 0), stop=(kk == C - 1))
                    drain(ps, boff, bw)

            def emit_proj_some(n):
                for _ in range(n):
                    if pstate["next"] < len(pblocks):
                        emit_proj_block()

            def emit_proj_upto(col_end):
                while (pstate["next"] < len(pblocks)
                       and pblocks[pstate["next"]][0] < col_end):
                    emit_proj_block()

            vstate = {"next": 0}

            def emit_v_block():
                boff, bw = pblocks[vstate["next"]]
                vstate["next"] += 1
                ps = pmm.tile([128, 512], F32, tag="mm")
                for kk in range(C):
                    nc.tensor.matmul(ps[:, 0:bw], wvs[kk][:],
                                     xt[kk][:, boff:boff + bw],
                                     start=(kk == 0), stop=(kk == C - 1))
                drain_v(ps, boff, bw)

            def emit_v_some(n):
                for _ in range(n):
                    if vstate["next"] < len(pblocks):
                        emit_v_block()

            def emit_v_upto(col_end):
                while (vstate["next"] < len(pblocks)
                       and pblocks[vstate["next"]][0] < col_end):
                    emit_v_block()

            # ---------------- diag products (q*k hi/lo), per 512 block -----
            prod_tiles = {}

            def emit_prod(e):
                off = offs[e]
                scb = lts[e] * 128
                for h in range(HL):
                    for bi, (boff, bw) in enumerate(blocks(scb, 512)):
                        sli = slice(off + boff, off + boff + bw)
                        pr32 = prp.tile([64, 512], F32, tag="pr32")
                        nc.vector.tensor_mul(pr32[:, 0:bw], qth[h][:, sli],
                                             kth[h][:, sli])
                        prh = prp.tile([64, 512], BF16, name=f"prh{h}_{bi}",
                                       tag=f"prh{h}_{bi}", bufs=1)
                        nc.gpsimd.tensor_copy(prh[:, 0:bw], pr32[:, 0:bw])
                        prl = prp.tile([64, 512], BF16, name=f"prl{h}_{bi}",
                                       tag=f"prl{h}_{bi}", bufs=1)
                        nc.gpsimd.tensor_sub(prl[:, 0:bw], pr32[:, 0:bw],
                                             prh[:, 0:bw])
                        prod_tiles[(e, h, bi)] = (prh, prl, bw)

            # ------------- epilogue of batch e as a unit queue -------------
            def epilogue_units(e, oeng):
                off = offs[e]
                lt = lts[e]
                scb = lt * 128
                st = {}

                def u_recip():
                    emit_v_upto(off + scb)
                    rec = prp.tile([128, 16], F32, tag="rec")
                    nc.vector.reciprocal(rec[:, 0:2 * lt], dn[e][:])
                    tpr = pmm.tile([16, 128], F32, tag="mm")
                    nc.tensor.transpose(tpr[0:2 * lt, :], rec[:, 0:2 * lt], idenf[:])
                    recT = prp.tile([16, 128], BF16, tag="recT")
                    nc.vector.tensor_copy(recT[0:2 * lt, :], tpr[0:2 * lt, :])
                    st["arecl"] = [dlp.tile([1, 1024], BF16, name=f"arecl{h}",
                                            tag=f"arecl{h}") for h in range(HL)]
                    for h in range(HL):
                        nc.sync.dma_start(st["arecl"][h][:, 0:scb],
                                          recT[h:2 * lt:2, :])
                yield u_recip

                def u_diag():
                    st["dexpl"] = [dlp.tile([1, 1024], BF16, name=f"dexpl{h}",
                                            tag=f"dexpl{h}") for h in range(HL)]
                    for h in range(HL):
                        for bi, (boff, bw) in enumerate(blocks(scb, 512)):
                            prh, prl, _ = prod_tiles.pop((e, h, bi))
                            dg = pmm.tile([1, 512], F32, tag="mm")
                            nc.tensor.matmul(dg[:, 0:bw], ones64[:], prh[:, 0:bw],
                                             start=True, stop=False)
                            nc.tensor.matmul(dg[:, 0:bw], ones64[:], prl[:, 0:bw],
                                             start=False, stop=True)
                            nc.scalar.activation(st["dexpl"][h][:, boff:boff + bw],
                                                 dg[:, 0:bw], AF.Exp)
                yield u_diag

                def u_a2():
                    for h in range(HL):
                        nc.vector.tensor_mul(a2[32 * h:32 * h + 1, off:off + scb],
                                             st["dexpl"][h][:, 0:scb],
                                             st["arecl"][h][:, 0:scb])
                yield u_a2

                for (boff, bw) in blocks(scb, 512):
                    def u_ab(boff=boff, bw=bw):
                        ab = pmm.tile([128, 512], F32, tag="mm")
                        nc.tensor.matmul(ab[:, 0:bw], ind2[:],
                                         a2[:, off + boff: off + boff + bw],
                                         start=True, stop=True)
                        nc.vector.tensor_mul(vt[:, off + boff: off + boff + bw],
                                             vt[:, off + boff: off + boff + bw],
                                             ab[:, 0:bw])
                    yield u_ab

                for i in range(lt):
                    def u_out(i=i):
                        cs = off + i * 128
                        ot = otp.tile([128, 1024], BF16, tag="ot")
                        for oi, (ooff, ow) in enumerate(blocks(D, 512)):
                            po = pmm.tile([128, 512], F32, tag="mm")
                            nc.tensor.matmul(po[:, 0:ow], vt[:, cs:cs + 128],
                                             w0s[:, ooff:ooff + ow],
                                             start=True, stop=True)
                            eng = oeng[0]
                            oeng[0] = (oeng[0] + 1) % 2
                            if eng == 0:
                                nc.vector.tensor_copy(ot[:, ooff:ooff + ow],
                                                      po[:, 0:ow])
                            else:
                                nc.scalar.copy(ot[:, ooff:ooff + ow], po[:, 0:ow])
                        nc.sync.dma_start(out_d[cs:cs + 128, :], ot[:])
                    yield u_out

            # ---------------- scores with woven epilogue ----------------
            def emit_scores(b, pending):
                off = offs[b]
                lt = lts[b]
                emit_proj_upto(off + lt * 128)
                for i in range(lt):
                    emit_proj_some(1)
                    emit_v_some(1)
                    N = (i + 1) * 128
                    if N <= 512:
                        # pack both heads in one PSUM tile at 512-aligned slot
                        # offsets (a matmul write must not cross a PSUM bank
                        # boundary): one exp + one 3-D reduce for the pair
                        w = N
                        sc = psc.tile([128, 1024], F32, tag="sc")
                        sc3 = sc.rearrange("p (s c) -> p s c", s=2)
                        for h in range(HL):
                            so = h * 512
                            nc.tensor.matmul(sc[:, so:so + w],
                                             qth[h][:, off + i * 128: off + N],
                                             kth[h][:, off: off + w],
                                             start=True, stop=False)
                            nc.tensor.matmul(sc[:, so + w - 128: so + w],
                                             negi[:], ub[:],
                                             start=False, stop=True,
                                             skip_group_check=True)
                        scr = srp.tile([128, 2, 512], BF16, tag="scr")
                        nc.scalar.activation(scr[:, 0:2, 0:w], sc3[:, 0:2, 0:w],
                                             AF.Exp)
                        nc.vector.tensor_reduce(dn[b][:, 2 * i: 2 * i + 2],
                                                scr[:, 0:2, 0:w], AX.X, ALU.add)
                    else:
                        for h in range(HL):
                            sc = psc.tile([128, 1024], F32, tag="sc")
                            bl = blocks(N, 512)
                            for bi, (boff, bw) in enumerate(bl):
                                last = (bi == len(bl) - 1)
                                nc.tensor.matmul(
                                    sc[:, boff:boff + bw],
                                    qth[h][:, off + i * 128: off + N],
                                    kth[h][:, off + boff: off + boff + bw],
                                    start=True, stop=not last)
                            nc.tensor.matmul(sc[:, N - 128:N], negi[:], ub[:],
                                             start=False, stop=True,
                                             skip_group_check=True)
                            scr = srp.tile([128, 1024], BF16, tag="scrw")
                            nc.scalar.activation(scr[:, 0:N], sc[:, 0:N], AF.Exp,
                                                 accum_out=dn[b][:, 2 * i + h:
                                                                 2 * i + h + 1])
                    for _ in range(3):
                        if pending:
                            pending.pop(0)()
                emit_prod(b)

            # ---------------- main schedule ----------------
            oeng = [0]
            pending = []
            for b in range(NB):
                emit_scores(b, pending)
                while pending:
                    pending.pop(0)()
                pending = list(epilogue_units(b, oeng))
            while pending:
                pending.pop(0)()

    nc.compile()
    return nc


def _get_nc(lts):
    key = tuple(lts)
    if key not in _CACHE:
        _CACHE[key] = _build(key)
    return _CACHE[key]


def _host_consts():
    aux = {}
    negi = np.zeros((128, 128), np.float32)
    np.fill_diagonal(negi, NEG)
    aux["negi"] = negi.astype(ml_dtypes.bfloat16)
    aux["ub"] = np.triu(np.ones((128, 128), np.float32), 1).astype(ml_dtypes.bfloat16)
    aux["idenf"] = np.eye(128, dtype=np.float32)
    aux["ones64"] = np.ones((64, 1), np.float32).astype(ml_dtypes.bfloat16)
    ind2 = np.zeros((33, 128), np.float32)
    ind2[0, 0:64] = 1.0
    ind2[32, 64:128] = 1.0
    aux["ind2"] = ind2.astype(ml_dtypes.bfloat16)
    return aux


def _run(inputs, trace=False):
    from concourse.bass_utils import run_bass_kernel_spmd

    batch = np.asarray(inputs["batch"], np.float32)
    lengths = np.asarray(inputs["lengths"]).astype(np.int64)
    assert batch.shape == (B, S, D), batch.shape
    lt_all = [max(1, int(np.ceil(int(l) / 128.0))) for l in lengths]
    order = sorted(range(B), key=lambda b: -lt_all[b])
    lts = tuple(lt_all[b] for b in order)
    offs = []
    o = 0
    for lt in lts:
        offs.append(o * 128)
        o += lt
    SC = o * 128

    nc = _get_nc(lts)

    XT = np.concatenate(
        [batch[order[k]][: lts[k] * 128, :].T for k in range(B)], axis=1)
    xt = np.ascontiguousarray(XT.reshape(D // 128, 128, SC)).astype(ml_dtypes.bfloat16)
    consts = _host_consts()
    wq = np.asarray(inputs["wq"], np.float32)
    wk = np.asarray(inputs["wk"], np.float32)
    wv = np.asarray(inputs["wv"], np.float32)
    w0 = np.asarray(inputs["w0"], np.float32)
    bqf = np.asarray(inputs["bq"], np.float32)
    bkf = np.asarray(inputs["bk"], np.float32)
    bvf = np.asarray(inputs["bv"], np.float32)

    in_maps = []
    for j in range(8):
        sl = slice(j * 128, (j + 1) * 128)
        im = dict(consts)
        im["xt"] = xt
        im["wq"] = np.ascontiguousarray(
            wq[:, sl].reshape(8, 128, 128)).astype(ml_dtypes.bfloat16)
        im["wk"] = np.ascontiguousarray(
            wk[:, sl].reshape(8, 128, 128)).astype(ml_dtypes.bfloat16)
        im["wv"] = np.ascontiguousarray(
            wv[:, sl].reshape(8, 128, 128)).astype(ml_dtypes.bfloat16)
        im["w0"] = np.ascontiguousarray(w0[sl, :]).astype(ml_dtypes.bfloat16)
        im["bq"] = np.ascontiguousarray(bqf[sl].reshape(128, 1))
        im["bk"] = np.ascontiguousarray(bkf[sl].reshape(128, 1))
        im["bv"] = np.ascontiguousarray(bvf[sl].reshape(128, 1))
        in_maps.append(im)

    res = run_bass_kernel_spmd(nc, in_maps, core_ids=list(range(8)), trace=trace)

    acc = np.zeros((SC, D), np.float32)
    for r in res.results:
        acc += np.asarray(r["out"]).astype(np.float32)
    b0 = np.asarray(inputs["b0"], np.float32)
    out = np.empty((B, S, D), np.float32)
    out[:] = b0[None, None, :]
    for k in range(B):
        b = order[k]
        L = int(lengths[b])
        out[b, :L, :] += acc[offs[k]: offs[k] + L, :]
    return out, res


def kernel(**inputs) -> np.ndarray:
    out, _ = _run(inputs, trace=False)
    return out


# revision 23
# speedup vs baseline: 1.2128x; 1.0360x over previous
"""Trainium2 Bass kernel for nn_MultiHeadAttention_85761906966848 (sparse_attention).

The reference module only uses the DIAGONAL of the softmax attention matrix:
    out[b,s,:] = (softmax(masked scores)[s,s] * v[b,s,:]) @ W0 + b0
so no attn @ V matmul is needed — only QK^T row-sums of exp (softmax
denominators), the diagonal q_s.k_s, and the four dense projections.

Sharding: TENSOR-PARALLEL over heads. Core j owns heads (2j, 2j+1):
  * Q/K/V projections restricted to that 128-wide feature slice for ALL
    batches, sequence axis trimmed to ceil(L_b/128)*128 valid rows.
  * Scores / softmax denominators / diagonal weights per local head.
  * O-projection uses the 128-row slice of W0 -> per-core PARTIAL outputs,
    summed on the host (linear combine), which also adds b0.
Uniform SPMD by construction; only the weight slices per in_map differ.

Matmuls in bf16; causal mask folded into the score matmul as an extra
(-1e30*I).T @ strict_upper_ones PSUM accumulation. Score tiles of width<=512
pack BOTH heads into one PSUM tile: one exp + one strided 3-D reduce per
pair, halving the scalar-engine op count on the critical softmax path.
Epilogue work of batch b-1 is woven between score tiles of batch b.
"""

import numpy as np
import ml_dtypes
import concourse.bass as bass
import concourse.bacc as bacc
import concourse.mybir as mybir
from concourse import tile

F32 = mybir.dt.float32
BF16 = mybir.dt.bfloat16
AF = mybir.ActivationFunctionType
AX = mybir.AxisListType
ALU = mybir.AluOpType

B, S, D, H = 8, 1024, 1024, 16
dk = D // H
HL = 2
NEG = -1.0e30

_CACHE = {}


def blocks(total, width):
    out = []
    off = 0
    while off < total:
        w = min(width, total - off)
        out.append((off, w))
        off += w
    return out


def _build(lts):
    """lts: tuple of per-batch 128-row tile counts, in processing order."""
    NB = len(lts)
    TC = sum(lts)
    SC = TC * 128
    offs = []
    o = 0
    for lt in lts:
        offs.append(o * 128)
        o += lt
    C = D // 128

    nc = bacc.Bacc("TRN2", target_bir_lowering=False, debug=False, num_devices=8)

    xt_d = nc.dram_tensor("xt", [C, 128, SC], BF16, kind="ExternalInput")
    wq_d = nc.dram_tensor("wq", [C, 128, 128], BF16, kind="ExternalInput")
    wk_d = nc.dram_tensor("wk", [C, 128, 128], BF16, kind="ExternalInput")
    wv_d = nc.dram_tensor("wv", [C, 128, 128], BF16, kind="ExternalInput")
    w0_d = nc.dram_tensor("w0", [128, D], BF16, kind="ExternalInput")
    bq_d = nc.dram_tensor("bq", [128, 1], F32, kind="ExternalInput")
    bk_d = nc.dram_tensor("bk", [128, 1], F32, kind="ExternalInput")
    bv_d = nc.dram_tensor("bv", [128, 1], F32, kind="ExternalInput")
    negi_d = nc.dram_tensor("negi", [128, 128], BF16, kind="ExternalInput")
    ub_d = nc.dram_tensor("ub", [128, 128], BF16, kind="ExternalInput")
    idenf_d = nc.dram_tensor("idenf", [128, 128], F32, kind="ExternalInput")
    ones64_d = nc.dram_tensor("ones64", [64, 1], BF16, kind="ExternalInput")
    ind2_d = nc.dram_tensor("ind2", [33, 128], BF16, kind="ExternalInput")
    out_d = nc.dram_tensor("out", [SC, D], BF16, kind="ExternalOutput")

    with tile.TileContext(nc) as tc:
        with (
            tc.tile_pool(name="cp", bufs=1) as cp,
            tc.tile_pool(name="xp", bufs=1) as xp,
            tc.tile_pool(name="qkp", bufs=1) as qkp,
            tc.tile_pool(name="srp", bufs=3) as srp,
            tc.tile_pool(name="prp", bufs=3) as prp,
            tc.tile_pool(name="dlp", bufs=2) as dlp,
            tc.tile_pool(name="otp", bufs=3) as otp,
            tc.tile_pool(name="psc", bufs=3, space=bass.MemorySpace.PSUM) as psc,
            tc.tile_pool(name="pmm", bufs=2, space=bass.MemorySpace.PSUM) as pmm,
        ):
            # ---------------- constants (sync queue) ----------------
            negi = cp.tile([128, 128], BF16, tag="negi")
            nc.sync.dma_start(negi[:], negi_d[:])
            ub = cp.tile([128, 128], BF16, tag="ub")
            nc.sync.dma_start(ub[:], ub_d[:])
            idenf = cp.tile([128, 128], F32, tag="idenf")
            nc.sync.dma_start(idenf[:], idenf_d[:])
            ones64 = cp.tile([64, 1], BF16, tag="ones64")
            nc.sync.dma_start(ones64[:], ones64_d[:])
            ind2 = cp.tile([33, 128], BF16, tag="ind2")
            nc.sync.dma_start(ind2[:], ind2_d[:])
            bq = cp.tile([128, 1], F32, tag="bq")
            nc.sync.dma_start(bq[:], bq_d[:])
            bk = cp.tile([128, 1], F32, tag="bk")
            nc.sync.dma_start(bk[:], bk_d[:])
            bv = cp.tile([128, 1], F32, tag="bv")
            nc.sync.dma_start(bv[:], bv_d[:])

            # weights on scalar queue first (small, needed first)
            wqs = [cp.tile([128, 128], BF16, name=f"wq{c}", tag=f"wq{c}") for c in range(C)]
            wks = [cp.tile([128, 128], BF16, name=f"wk{c}", tag=f"wk{c}") for c in range(C)]
            wvs = [cp.tile([128, 128], BF16, name=f"wv{c}", tag=f"wv{c}") for c in range(C)]
            for c in range(C):
                nc.scalar.dma_start(wks[c][:], wk_d[c, :, :])
            for c in range(C):
                nc.scalar.dma_start(wqs[c][:], wq_d[c, :, :])

            # X^T resident: narrow first block for a fast start, wide after
            xt = [xp.tile([128, SC], BF16, name=f"xt{c}", tag=f"xt{c}") for c in range(C)]
            di = 0
            xengs = [nc.sync, nc.scalar, nc.gpsimd]
            for (boff, bw) in blocks(SC, 1024):
                for c in range(C):
                    xengs[di % 3].dma_start(xt[c][:, boff:boff + bw],
                                            xt_d[c, :, boff:boff + bw])
                    di += 1
            for c in range(C):
                nc.scalar.dma_start(wvs[c][:], wv_d[c, :, :])
            w0s = cp.tile([128, D], BF16, tag="w0s")
            nc.scalar.dma_start(w0s[:], w0_d[:])

            # persistent per-core tensors
            qth = [qkp.tile([64, SC], BF16, name=f"qth{h}", tag=f"qth{h}") for h in range(HL)]
            kth = [qkp.tile([64, SC], BF16, name=f"kth{h}", tag=f"kth{h}") for h in range(HL)]
            vt = qkp.tile([128, SC], BF16, tag="vt")
            a2 = cp.tile([33, SC], BF16, tag="a2")
            nc.vector.memset(a2[:], 0.0)
            dn = [cp.tile([128, 2 * lt], F32, name=f"dn{b}", tag=f"dn{b}")
                  for b, lt in enumerate(lts)]

            # -------- streaming K/Q/V projection, one 512 block at a time
            def drain_k(ps, boff, bw):
                nc.scalar.activation(kth[0][:, boff:boff + bw], ps[0:64, 0:bw],
                                     AF.Identity, bias=bk[0:64, :])
                nc.vector.tensor_scalar_add(kth[1][:, boff:boff + bw],
                                            ps[64:128, 0:bw], bk[64:128, :])

            def drain_q(ps, boff, bw):
                nc.vector.tensor_scalar_add(qth[0][:, boff:boff + bw],
                                            ps[0:64, 0:bw], bq[0:64, :])
                nc.scalar.activation(qth[1][:, boff:boff + bw], ps[64:128, 0:bw],
                                     AF.Identity, bias=bq[64:128, :])

            def drain_v(ps, boff, bw):
                nc.vector.tensor_scalar_add(vt[:, boff:boff + bw], ps[:, 0:bw],
                                            bv[:])

            pblocks = blocks(SC, 512)
            pstate = {"next": 0}

            def emit_proj_block():
                # consume a pair of 512 blocks through one [128,1024] PSUM
                # tile (bank-aligned halves, contiguous since half0 is full):
                # halves the chain->drain->release round-trips
                pair = []
                while len(pair) < 2 and pstate["next"] < len(pblocks):
                    pair.append(pblocks[pstate["next"]])
                    pstate["next"] += 1
                boff = pair[0][0]
                tw = sum(w for (_, w) in pair)
                for w_tiles, drain in ((wks, drain_k), (wqs, drain_q)):
                    ps = psc.tile([128, 1024], F32, tag="sc")
                    for hi, (po, pw) in enumerate(pair):
                        so = hi * 512
                        for kk in range(C):
                            nc.tensor.matmul(ps[:, so:so + pw], w_tiles[kk][:],
                                             xt[kk][:, po:po + pw],
                                             start=(kk == 0), stop=(kk == C - 1))
                    drain(ps, boff, tw)

            def emit_proj_some(n):
                for _ in range(n):
                    if pstate["next"] < len(pblocks):
                        emit_proj_block()

            def emit_proj_upto(col_end):
                while (pstate["next"] < len(pblocks)
                       and pblocks[pstate["next"]][0] < col_end):
                    emit_proj_block()

            vstate = {"next": 0}

            def emit_v_block():
                boff, bw = pblocks[vstate["next"]]
                vstate["next"] += 1
                ps = pmm.tile([128, 512], F32, tag="mm")
                for kk in range(C):
                    nc.tensor.matmul(ps[:, 0:bw], wvs[kk][:],
                                     xt[kk][:, boff:boff + bw],
                                     start=(kk == 0), stop=(kk == C - 1))
                drain_v(ps, boff, bw)

            def emit_v_some(n):
                for _ in range(n):
                    if vstate["next"] < len(pblocks):
                        emit_v_block()

            def emit_v_upto(col_end):
                while (vstate["next"] < len(pblocks)
                       and pblocks[vstate["next"]][0] < col_end):
                    emit_v_block()

            # ---------------- diag products (q*k hi/lo), per 512 block -----
            prod_tiles = {}

            def emit_prod(e):
                off = offs[e]
                scb = lts[e] * 128
                for h in range(HL):
                    for bi, (boff, bw) in enumerate(blocks(scb, 512)):
                        sli = slice(off + boff, off + boff + bw)
                        pr32 = prp.tile([64, 512], F32, tag="pr32")
                        nc.vector.tensor_mul(pr32[:, 0:bw], qth[h][:, sli],
                                             kth[h][:, sli])
                        prh = prp.tile([64, 512], BF16, name=f"prh{h}_{bi}",
                                       tag=f"prh{h}_{bi}", bufs=1)
                        nc.gpsimd.tensor_copy(prh[:, 0:bw], pr32[:, 0:bw])
                        prl = prp.tile([64, 512], BF16, name=f"prl{h}_{bi}",
                                       tag=f"prl{h}_{bi}", bufs=1)
                        nc.gpsimd.tensor_sub(prl[:, 0:bw], pr32[:, 0:bw],
                                             prh[:, 0:bw])
                        prod_tiles[(e, h, bi)] = (prh, prl, bw)

            # ------------- epilogue of batch e as a unit queue -------------
            def epilogue_units(e, oeng):
                off = offs[e]
                lt = lts[e]
                scb = lt * 128
                st = {}

                def u_recip():
                    emit_v_upto(off + scb)
                    rec = prp.tile([128, 16], F32, tag="rec")
                    nc.vector.reciprocal(rec[:, 0:2 * lt], dn[e][:])
                    tpr = pmm.tile([16, 128], F32, tag="mm")
                    nc.tensor.transpose(tpr[0:2 * lt, :], rec[:, 0:2 * lt], idenf[:])
                    recT = prp.tile([16, 128], BF16, tag="recT")
                    nc.vector.tensor_copy(recT[0:2 * lt, :], tpr[0:2 * lt, :])
                    st["arecl"] = [dlp.tile([1, 1024], BF16, name=f"arecl{h}",
                                            tag=f"arecl{h}") for h in range(HL)]
                    for h in range(HL):
                        nc.sync.dma_start(st["arecl"][h][:, 0:scb],
                                          recT[h:2 * lt:2, :])
                yield u_recip

                def u_diag():
                    st["dexpl"] = [dlp.tile([1, 1024], BF16, name=f"dexpl{h}",
                                            tag=f"dexpl{h}") for h in range(HL)]
                    for h in range(HL):
                        for bi, (boff, bw) in enumerate(blocks(scb, 512)):
                            prh, prl, _ = prod_tiles.pop((e, h, bi))
                            dg = pmm.tile([1, 512], F32, tag="mm")
                            nc.tensor.matmul(dg[:, 0:bw], ones64[:], prh[:, 0:bw],
                                             start=True, stop=False)
                            nc.tensor.matmul(dg[:, 0:bw], ones64[:], prl[:, 0:bw],
                                             start=False, stop=True)
                            nc.scalar.activation(st["dexpl"][h][:, boff:boff + bw],
                                                 dg[:, 0:bw], AF.Exp)
                yield u_diag

                def u_a2():
                    for h in range(HL):
                        nc.vector.tensor_mul(a2[32 * h:32 * h + 1, off:off + scb],
                                             st["dexpl"][h][:, 0:scb],
                                             st["arecl"][h][:, 0:scb])
                yield u_a2

                for (boff, bw) in blocks(scb, 512):
                    def u_ab(boff=boff, bw=bw):
                        ab = pmm.tile([128, 512], F32, tag="mm")
                        nc.tensor.matmul(ab[:, 0:bw], ind2[:],
                                         a2[:, off + boff: off + boff + bw],
                                         start=True, stop=True)
                        nc.vector.tensor_mul(vt[:, off + boff: off + boff + bw],
                                             vt[:, off + boff: off + boff + bw],
                                             ab[:, 0:bw])
                    yield u_ab

                for i in range(lt):
                    def u_out(i=i):
                        cs = off + i * 128
                        ot = otp.tile([128, 1024], BF16, tag="ot")
                        for oi, (ooff, ow) in enumerate(blocks(D, 512)):
                            po = pmm.tile([128, 512], F32, tag="mm")
                            nc.tensor.matmul(po[:, 0:ow], vt[:, cs:cs + 128],
                                             w0s[:, ooff:ooff + ow],
                                             start=True, stop=True)
                            eng = oeng[0]
                            oeng[0] = (oeng[0] + 1) % 2
                            if eng == 0:
                                nc.vector.tensor_copy(ot[:, ooff:ooff + ow],
                                                      po[:, 0:ow])
                            else:
                                nc.scalar.copy(ot[:, ooff:ooff + ow], po[:, 0:ow])
                        nc.sync.dma_start(out_d[cs:cs + 128, :], ot[:])
                    yield u_out

            # ---------------- scores with woven epilogue ----------------
            def emit_scores(b, pending):
                off = offs[b]
                lt = lts[b]
                emit_proj_upto(off + lt * 128)
                for i in range(lt):
                    emit_proj_some(1)
                    emit_v_some(1)
                    N = (i + 1) * 128
                    if N <= 512:
                        # pack both heads in one PSUM tile at 512-aligned slot
                        # offsets (a matmul write must not cross a PSUM bank
                        # boundary): one exp + one 3-D reduce for the pair
                        w = N
                        sc = psc.tile([128, 1024], F32, tag="sc")
                        sc3 = sc.rearrange("p (s c) -> p s c", s=2)
                        for h in range(HL):
                            so = h * 512
                            nc.tensor.matmul(sc[:, so:so + w],
                                             qth[h][:, off + i * 128: off + N],
                                             kth[h][:, off: off + w],
                                             start=True, stop=False)
                            nc.tensor.matmul(sc[:, so + w - 128: so + w],
                                             negi[:], ub[:],
                                             start=False, stop=True,
                                             skip_group_check=True)
                        scr = srp.tile([128, 2, 512], BF16, tag="scr")
                        nc.scalar.activation(scr[:, 0:2, 0:w], sc3[:, 0:2, 0:w],
                                             AF.Exp)
                        nc.vector.tensor_reduce(dn[b][:, 2 * i: 2 * i + 2],
                                                scr[:, 0:2, 0:w], AX.X, ALU.add)
                    else:
                        for h in range(HL):
                            sc = psc.tile([128, 1024], F32, tag="sc")
                            bl = blocks(N, 512)
                            for bi, (boff, bw) in enumerate(bl):
                                last = (bi == len(bl) - 1)
                                nc.tensor.matmul(
                                    sc[:, boff:boff + bw],
                                    qth[h][:, off + i * 128: off + N],
                                    kth[h][:, off + boff: off + boff + bw],
                                    start=True, stop=not last)
                            nc.tensor.matmul(sc[:, N - 128:N], negi[:], ub[:],
                                             start=False, stop=True,
                                             skip_group_check=True)
                            scr = srp.tile([128, 1024], BF16, tag="scrw")
                            nc.scalar.activation(scr[:, 0:N], sc[:, 0:N], AF.Exp,
                                                 accum_out=dn[b][:, 2 * i + h:
                                                                 2 * i + h + 1])
                    for _ in range(3):
                        if pending:
                            pending.pop(0)()
                emit_prod(b)

            # ---------------- main schedule ----------------
            oeng = [0]
            pending = []
            for b in range(NB):
                emit_scores(b, pending)
                while pending:
                    pending.pop(0)()
                pending = list(epilogue_units(b, oeng))
            while pending:
                pending.pop(0)()

    nc.compile()
    return nc


def _get_nc(lts):
    key = tuple(lts)
    if key not in _CACHE:
        _CACHE[key] = _build(key)
    return _CACHE[key]


def _host_consts():
    aux = {}
    negi = np.zeros((128, 128), np.float32)
    np.fill_diagonal(negi, NEG)
    aux["negi"] = negi.astype(ml_dtypes.bfloat16)
    aux["ub"] = np.triu(np.ones((128, 128), np.float32), 1).astype(ml_dtypes.bfloat16)
    aux["idenf"] = np.eye(128, dtype=np.float32)
    aux["ones64"] = np.ones((64, 1), np.float32).astype(ml_dtypes.bfloat16)
    ind2 = np.zeros((33, 128), np.float32)
    ind2[0, 0:64] = 1.0
    ind2[32, 64:128] = 1.0
    aux["ind2"] = ind2.astype(ml_dtypes.bfloat16)
    return aux


def _run(inputs, trace=False):
    from concourse.bass_utils import run_bass_kernel_spmd

    batch = np.asarray(inputs["batch"], np.float32)
    lengths = np.asarray(inputs["lengths"]).astype(np.int64)
    assert batch.shape == (B, S, D), batch.shape
    lt_all = [max(1, int(np.ceil(int(l) / 128.0))) for l in lengths]
    order = sorted(range(B), key=lambda b: -lt_all[b])
    lts = tuple(lt_all[b] for b in order)
    offs = []
    o = 0
    for lt in lts:
        offs.append(o * 128)
        o += lt
    SC = o * 128

    nc = _get_nc(lts)

    XT = np.concatenate(
        [batch[order[k]][: lts[k] * 128, :].T for k in range(B)], axis=1)
    xt = np.ascontiguousarray(XT.reshape(D // 128, 128, SC)).astype(ml_dtypes.bfloat16)
    consts = _host_consts()
    wq = np.asarray(inputs["wq"], np.float32)
    wk = np.asarray(inputs["wk"], np.float32)
    wv = np.asarray(inputs["wv"], np.float32)
    w0 = np.asarray(inputs["w0"], np.float32)
    bqf = np.asarray(inputs["bq"], np.float32)
    bkf = np.asarray(inputs["bk"], np.float32)
    bvf = np.asarray(inputs["bv"], np.float32)

    in_maps = []
    for j in range(8):
        sl = slice(j * 128, (j + 1) * 128)
        im = dict(consts)
        im["xt"] = xt
        im["wq"] = np.ascontiguousarray(
            wq[:, sl].reshape(8, 128, 128)).astype(ml_dtypes.bfloat16)
        im["wk"] = np.ascontiguousarray(
            wk[:, sl].reshape(8, 128, 128)).astype(ml_dtypes.bfloat16)
        im["wv"] = np.ascontiguousarray(
            wv[:, sl].reshape(8, 128, 128)).astype(ml_dtypes.bfloat16)
        im["w0"] = np.ascontiguousarray(w0[sl, :]).astype(ml_dtypes.bfloat16)
        im["bq"] = np.ascontiguousarray(bqf[sl].reshape(128, 1))
        im["bk"] = np.ascontiguousarray(bkf[sl].reshape(128, 1))
        im["bv"] = np.ascontiguousarray(bvf[sl].reshape(128, 1))
        in_maps.append(im)

    res = run_bass_kernel_spmd(nc, in_maps, core_ids=list(range(8)), trace=trace)

    acc = np.zeros((SC, D), np.float32)
    for r in res.results:
        acc += np.asarray(r["out"]).astype(np.float32)
    b0 = np.asarray(inputs["b0"], np.float32)
    out = np.empty((B, S, D), np.float32)
    out[:] = b0[None, None, :]
    for k in range(B):
        b = order[k]
        L = int(lengths[b])
        out[b, :L, :] += acc[offs[k]: offs[k] + L, :]
    return out, res


def kernel(**inputs) -> np.ndarray:
    out, _ = _run(inputs, trace=False)
    return out
